# revision 24
# baseline (speedup 1.0000x reference)
import sys

sys.path.insert(0, "/opt/trn_rl_repo")

import numpy as np
import ml_dtypes

import concourse.bass as bass
import concourse.mybir as mybir
import concourse.tile as tile
from concourse import bacc
from concourse import bass_utils

F32 = mybir.dt.float32
F32R = mybir.dt.float32r
BF16 = mybir.dt.bfloat16


def _r(ap):
    return ap.bitcast(F32R)


def _round_f32r(a):
    # round-to-nearest-even fp32 -> fp32r (sign+8exp+11mant in top 20 bits)
    u = np.ascontiguousarray(a, np.float32).view(np.uint32)
    bias = np.uint32(0x7FF) + ((u >> np.uint32(12)) & np.uint32(1))
    u2 = ((u + bias) >> np.uint32(12)) << np.uint32(12)
    return u2.view(np.float32)

B, S, D, H, DK = 4, 2048, 512, 8, 64
NCORES = 8
SQ = S // 2          # queries per core (q-half sharding)
QC = SQ // 128       # 8 q-chunks of 128
KT = S // 512        # 4 k-tiles of 512
IC = D // 128        # 4 contraction chunks
OC = D // 128        # 4 output chunks (head-pairs)
NEG = -1.0e9
EPS = 1.0e-5


def _cp(nc, i, out, in_):
    # alternate PSUM->SBUF evacuations between DVE and ACT
    if i % 2 == 0:
        nc.vector.tensor_copy(out=out, in_=in_)
    else:
        nc.scalar.copy(out=out, in_=in_)


def _emit(tc, nc, io):
    from contextlib import ExitStack

    with ExitStack() as ctx:
        persist = ctx.enter_context(tc.tile_pool(name="persist", bufs=1))

        # --- persistent tiles ---
        ident = persist.tile([128, 128], F32, tag="ident")
        nc.sync.dma_start(ident[:], io["identity"][:])
        diag = persist.tile([128, 128], BF16, tag="diag")
        nc.sync.dma_start(diag[:], io["diag_neg"][:])
        qT_sb = persist.tile([128, OC, SQ], F32R, tag="qT")
        kT_sb = persist.tile([128, OC, S], F32R, tag="kT")
        v_sb = persist.tile([128, S // 128, D], F32R, tag="v")
        probT_sb = persist.tile([64, H, SQ], F32R, tag="probT")

        # --- phase 1: projections ---
        def proj_dT(w_name, x_name, out_sb, sfree):
            # out.T[o, s] tiles: lhsT = w.T [i,o] blocks, rhs = x.T [i, s]
            with tc.tile_pool(name=f"p_{w_name}", bufs=1) as pp, \
                 tc.tile_pool(name=f"ps_{w_name}", bufs=4, space="PSUM") as pps:
                w_sb = pp.tile([128, IC, D], F32R, tag="w")
                nc.sync.dma_start(w_sb[:], io[w_name].rearrange("(ic p) o -> p ic o", p=128))
                x_sb = pp.tile([128, IC, sfree], F32R, tag="x")
                nc.sync.dma_start(x_sb[:], io[x_name].rearrange("(ic p) s -> p ic s", p=128))
                for oc in range(OC):
                    for st in range(sfree // 512):
                        ps = pps.tile([128, 512], F32, tag="ps")
                        for ic in range(IC):
                            nc.tensor.matmul(
                                ps[:],
                                w_sb[:, ic, oc * 128:(oc + 1) * 128],
                                x_sb[:, ic, st * 512:(st + 1) * 512],
                                start=(ic == 0), stop=(ic == IC - 1),
                            )
                        _cp(nc, oc + st, out_sb[:, oc, st * 512:(st + 1) * 512], ps[:])

        proj_dT("wqT", "xqT", qT_sb, SQ)
        proj_dT("wkT", "xkT", kT_sb, S)

        # V natural [s, o]: lhsT = xvT [i, s] blocks, rhs = wvT [i, o]
        with tc.tile_pool(name="p_v", bufs=1) as pp, \
             tc.tile_pool(name="ps_v", bufs=4, space="PSUM") as pps:
            w_sb = pp.tile([128, IC, D], F32R, tag="w")
            nc.sync.dma_start(w_sb[:], io["wvT"].rearrange("(ic p) o -> p ic o", p=128))
            x_sb = pp.tile([128, IC, S], F32R, tag="x")
            nc.sync.dma_start(x_sb[:], io["xvT"].rearrange("(ic p) s -> p ic s", p=128))
            for sc in range(S // 128):
                ps = pps.tile([128, 512], F32, tag="ps")
                for ic in range(IC):
                    nc.tensor.matmul(
                        ps[:],
                        x_sb[:, ic, sc * 128:(sc + 1) * 128],
                        w_sb[:, ic, :],
                        start=(ic == 0), stop=(ic == IC - 1),
                    )
                _cp(nc, sc, v_sb[:, sc, :], ps[:])

        # --- phase 2: attention ---
        with tc.tile_pool(name="attn_sb", bufs=7) as pa, \
             tc.tile_pool(name="small_sb", bufs=3) as psm, \
             tc.tile_pool(name="t_sb", bufs=3) as pt, \
             tc.tile_pool(name="mask_sb", bufs=1) as pm, \
             tc.tile_pool(name="ps_s", bufs=2, space="PSUM") as pss, \
             tc.tile_pool(name="ps_t", bufs=2, space="PSUM") as pst, \
             tc.tile_pool(name="ps_av", bufs=2, space="PSUM") as psa:

            mask_sb = pm.tile([128, QC, S], BF16, tag="mask")
            nc.sync.dma_start(mask_sb[:], io["mask"].rearrange("(qc p) k -> p qc k", p=128))

            NG = H * 2              # 16 groups of 4 q-chunks (one qt each)
            attn_live = {}          # (g, qj) -> attn tile
            av_tiles = {}           # g -> accum psum tile [64, 512]

            def produce(i):
                h, qc = i // QC, i % QC
                hp, hr = h // 2, (h % 2) * 64
                at = pa.tile([128, S], F32, tag="attn", name=f"attn_{i}")
                dsum = psm.tile([128, 4], F32, tag="dsum", name=f"dsum_{i}")
                for half in range(2):
                    sps = pss.tile([128, 1024], F32, tag="sps", name=f"sps_{i}_{half}")
                    for j in range(2):
                        kt = half * 2 + j
                        nc.tensor.matmul(
                            sps[:, j * 512:(j + 1) * 512],
                            qT_sb[hr:hr + 64, hp, qc * 128:(qc + 1) * 128],
                            kT_sb[hr:hr + 64, hp, kt * 512:(kt + 1) * 512],
                            start=True, stop=False,
                        )
                        nc.tensor.matmul(
                            sps[:, j * 512:(j + 1) * 512],
                            diag[:],
                            mask_sb[:, qc, kt * 512:(kt + 1) * 512],
                            start=False, stop=True,
                        )
                    nc.scalar.activation(
                        out=at[:, half * 1024:(half + 1) * 1024],
                        in_=sps[:],
                        func=mybir.ActivationFunctionType.Exp,
                        accum_out=dsum[:, half:half + 1],
                    )
                nc.vector.tensor_add(out=dsum[:, 2:3], in0=dsum[:, 0:1], in1=dsum[:, 1:2])
                nc.vector.reciprocal(out=dsum[:, 3:4], in_=dsum[:, 2:3])
                nc.vector.tensor_scalar_mul(out=at[:], in0=at[:], scalar1=dsum[:, 3:4])
                nc.sync.dma_start(io["attn_part"][h, qc * 128:(qc + 1) * 128, :], at[:])
                attn_live[(i // 4, i % 4)] = at

            def sweep(g, sub):
                # transpose 4 kc-chunks of group g and feed the AV matmul
                h, qt = g // 2, g % 2
                if sub == 0:
                    av_tiles[g] = psa.tile([64, 512], F32, tag="av", name=f"av_{g}")
                for kc in range(sub * 4, sub * 4 + 4):
                    tps = pst.tile([128, 512], F32, tag="tps", name=f"tps_{g}_{kc}")
                    for qj in range(4):
                        at = attn_live[(g, qj)]
                        nc.tensor.transpose(
                            tps[:, qj * 128:(qj + 1) * 128],
                            at[:, kc * 128:(kc + 1) * 128],
                            ident[:],
                        )
                    tsb = pt.tile([128, 512], F32R, tag="tsb", name=f"tsb_{g}_{kc}")
                    _cp(nc, kc, tsb[:], tps[:])
                    nc.tensor.matmul(
                        av_tiles[g][:],
                        v_sb[:, kc, h * 64:(h + 1) * 64],
                        tsb[:],
                        start=(kc == 0), stop=(kc == S // 128 - 1),
                    )
                if sub == 3:
                    _cp(nc, g, probT_sb[:, h, qt * 512:(qt + 1) * 512], av_tiles[g][:])
                    for qj in range(4):
                        del attn_live[(g, qj)]
                    del av_tiles[g]

            for i in range(H * QC):
                produce(i)
                if i // 4 >= 1:
                    sweep(i // 4 - 1, i % 4)
            for sub in range(4):
                sweep(NG - 1, sub)

        # --- phase 3: output projection + residual + layernorm ---
        with tc.tile_pool(name="ln_sb", bufs=1) as pl, \
             tc.tile_pool(name="ln_work", bufs=4) as pw, \
             tc.tile_pool(name="ps_o", bufs=4, space="PSUM") as pso:
            woT_sb = pl.tile([64, H, D], F32R, tag="woT")
            nc.sync.dma_start(woT_sb[:], io["woT"][:])
            xq_sb = pl.tile([128, QC, D], F32, tag="xq")
            nc.sync.dma_start(xq_sb[:], io["xq"].rearrange("(qc p) d -> p qc d", p=128))
            gam_sb = pl.tile([128, D], F32, tag="gam")
            nc.gpsimd.dma_start(out=gam_sb[:], in_=io["gamma"].to_broadcast((128, D)))
            bet_sb = pl.tile([128, D], F32, tag="bet")
            nc.gpsimd.dma_start(out=bet_sb[:], in_=io["beta"].to_broadcast((128, D)))
            eps_sb = pl.tile([128, 1], F32, tag="eps")
            nc.vector.memset(eps_sb[:], EPS)

            for qc in range(QC):
                ops = pso.tile([128, 512], F32, tag="ops")
                for h in range(H):
                    nc.tensor.matmul(
                        ops[:],
                        probT_sb[:, h, qc * 128:(qc + 1) * 128],
                        woT_sb[:, h, :],
                        start=(h == 0), stop=(h == H - 1),
                    )
                y = pw.tile([128, D], F32, tag="y")
                nc.vector.tensor_add(out=y[:], in0=ops[:], in1=xq_sb[:, qc, :])
                st = pw.tile([128, 6], F32, tag="st")
                nc.vector.bn_stats(out=st[:], in_=y[:])
                mv = pw.tile([128, 2], F32, tag="mv")
                nc.vector.bn_aggr(out=mv[:], in_=st[:])
                # rstd = 1/sqrt(var+eps)
                nc.scalar.activation(
                    out=mv[:, 1:2], in_=mv[:, 1:2],
                    func=mybir.ActivationFunctionType.Sqrt,
                    bias=eps_sb[:],
                )
                nc.vector.reciprocal(out=mv[:, 1:2], in_=mv[:, 1:2])
                nc.vector.tensor_scalar(
                    out=y[:], in0=y[:],
                    scalar1=mv[:, 0:1], scalar2=mv[:, 1:2],
                    op0=mybir.AluOpType.subtract, op1=mybir.AluOpType.mult,
                )
                nc.vector.tensor_mul(out=y[:], in0=y[:], in1=gam_sb[:])
                nc.vector.tensor_add(out=y[:], in0=y[:], in1=bet_sb[:])
                nc.sync.dma_start(io["out_part"][qc * 128:(qc + 1) * 128, :], y[:])


_CACHE = {}


def _build(reps=1):
    key = ("nc", reps)
    if key in _CACHE:
        return _CACHE[key]
    nc = bacc.Bacc("TRN2", target_bir_lowering=False, debug=False, num_devices=NCORES)
    io = {}
    io["xqT"] = nc.dram_tensor("xqT", [D, SQ], F32R, kind="ExternalInput").ap()
    io["xkT"] = nc.dram_tensor("xkT", [D, S], F32R, kind="ExternalInput").ap()
    io["xvT"] = nc.dram_tensor("xvT", [D, S], F32R, kind="ExternalInput").ap()
    io["xq"] = nc.dram_tensor("xq", [SQ, D], F32, kind="ExternalInput").ap()
    io["wqT"] = nc.dram_tensor("wqT", [D, D], F32R, kind="ExternalInput").ap()
    io["wkT"] = nc.dram_tensor("wkT", [D, D], F32R, kind="ExternalInput").ap()
    io["wvT"] = nc.dram_tensor("wvT", [D, D], F32R, kind="ExternalInput").ap()
    io["woT"] = nc.dram_tensor("woT", [64, H, D], F32R, kind="ExternalInput").ap()
    io["mask"] = nc.dram_tensor("mask", [SQ, S], BF16, kind="ExternalInput").ap()
    io["gamma"] = nc.dram_tensor("gamma", [1, D], F32, kind="ExternalInput").ap()
    io["beta"] = nc.dram_tensor("beta", [1, D], F32, kind="ExternalInput").ap()
    io["identity"] = nc.dram_tensor("identity", [128, 128], F32, kind="ExternalInput").ap()
    io["diag_neg"] = nc.dram_tensor("diag_neg", [128, 128], BF16, kind="ExternalInput").ap()
    io["attn_part"] = nc.dram_tensor("attn_part", [H, SQ, S], F32, kind="ExternalOutput").ap()
    io["out_part"] = nc.dram_tensor("out_part", [SQ, D], F32, kind="ExternalOutput").ap()
    with tile.TileContext(nc) as tc:
        for _ in range(reps):
            _emit(tc, nc, io)
    nc.compile()
    _CACHE[key] = nc
    return nc


def kernel(input_Q=None, input_K=None, input_V=None, attn_mask=None,
           W_Q=None, W_K=None, W_V=None, W_O=None, ln_gamma=None, ln_beta=None,
           _trace=False, _tracedir=None):
    input_Q = np.asarray(input_Q, np.float32)
    input_K = np.asarray(input_K, np.float32)
    input_V = np.asarray(input_V, np.float32)
    attn_mask = np.asarray(attn_mask)
    W_Q = np.asarray(W_Q, np.float32)
    W_K = np.asarray(W_K, np.float32)
    W_V = np.asarray(W_V, np.float32)
    W_O = np.asarray(W_O, np.float32)
    ln_gamma = np.asarray(ln_gamma, np.float32)
    ln_beta = np.asarray(ln_beta, np.float32)

    nc = _build(1)

    wqT = _round_f32r(W_Q.T / np.float32(np.sqrt(DK)))
    wkT = _round_f32r(W_K.T)
    wvT = _round_f32r(W_V.T)
    woT = _round_f32r(np.ascontiguousarray(W_O.T.reshape(H, 64, D).transpose(1, 0, 2)))
    ident = np.eye(128, dtype=np.float32)
    diag_neg = (np.eye(128) * NEG).astype(ml_dtypes.bfloat16)

    in_maps = []
    for c in range(NCORES):
        b, qh = c // 2, c % 2
        qs = slice(qh * SQ, (qh + 1) * SQ)
        in_maps.append({
            "xqT": _round_f32r(input_Q[b, qs].T),
            "xkT": _round_f32r(input_K[b].T),
            "xvT": _round_f32r(input_V[b].T),
            "xq": np.ascontiguousarray(input_Q[b, qs]),
            "wqT": wqT, "wkT": wkT, "wvT": wvT, "woT": woT,
            "mask": attn_mask[b, qs].astype(ml_dtypes.bfloat16),
            "gamma": ln_gamma.reshape(1, D), "beta": ln_beta.reshape(1, D),
            "identity": ident, "diag_neg": diag_neg,
        })

    res = bass_utils.run_bass_kernel_spmd(
        nc, in_maps, core_ids=list(range(NCORES)),
        trace=_trace, tmpdir=_tracedir,
    )
    if _trace:
        _CACHE["exec_time_ns"] = res.exec_time_ns
        _CACHE["results_obj"] = res

    attn = np.empty((B, H, S, S), np.float32)
    out = np.empty((B, S, D), np.float32)
    for c in range(NCORES):
        b, qh = c // 2, c % 2
        qs = slice(qh * SQ, (qh + 1) * SQ)
        attn[b, :, qs, :] = res.results[c]["attn_part"]
        out[b, qs, :] = res.results[c]["out_part"]
    return out, attn
